# revision 6
# baseline (speedup 1.0000x reference)
"""Trainium2 Bass kernel: 7x7 valid 2D cross-correlation of an 8192x8192
fp32 image plus scalar bias, row-sharded across 8 NeuronCores.

Formulation (per core): the y-direction 7-tap convolution for a fixed kernel
column dx is a banded matmul: out_dx[y, x] = sum_r A_dx[r, y] * X[r, x] with
A_dx[r, y] = K[r - y, dx].  The full conv accumulates the 7 dx terms in PSUM
with the moving operand (image columns) shifted by dx.  Matmuls run in bf16
(inputs bf16, fp32 PSUM accumulate); the banded weight blocks are padded to
128 columns so the compiler's fast-weight-load path engages.  This shape is
PE-bound at ~854 useful MACs/cycle (band 7 of a 128-deep contraction), which
is the provable ceiling for conv-as-banded-matmul on this array.

Work distribution: 8186 output rows = 68 bands of <=122 rows.  Each core gets
8 full bands (rows 976*i .. 976*i+976) plus HALF of one of bands 64..67
(8 column tiles), i.e. 136 (band, col-tile) units/core instead of the naive
9 full bands = 144 — the PE-time quantum is a full 512-column matmul pass, so
a 9th, mostly-empty band wastes 8 passes.

DMA plan (from traced ring behavior): the gpsimd SWDGE ring spreads one
DMA's rows over all 16 SDMA engines (fast); sync/scalar HWDGE rings serialize
~0.6us per 16 KB row on a single engine, and every ring serves entries in
order, with an entry's semaphore wait blocking everything behind it.  So all
loads and the bulk of stores ride gpsimd; the HWDGE rings get small store
slices.  Three half-band col tiles run FIRST (gated only on ~0.6 MB, so the
PE starts within ~3 us of the rings coming up), band 0's input arrives as
four quarter-width tiles that unblock its col-tile groups progressively,
and five half-band tiles run LAST (their input resident since band 0) so
the final main band's 2 MB store flush overlaps their compute.  Rings are
per-DMA-latency-bound during the ramp (~2 us per entry regardless of size),
and output tiles must stay band-wide: splitting them for progressive stores
serializes Tile's buffer sync into the PE stream (+24 us measured).
"""

import numpy as np
import ml_dtypes

import concourse.bass as bass
import concourse.mybir as mybir
from concourse.tile import TileContext
from concourse.bass_utils import run_bass_kernel_spmd

H = W = 8192
KH = KW = 7
OH = OW = H - KH + 1          # 8186
N_CORES = 8
BAND_IN = 128                 # input rows per matmul band (partition dim)
BAND_OUT = BAND_IN - KH + 1   # 122 output rows per band
APAD = 128                    # A block columns (padded from BAND_OUT for FWL)
COL_TILE = 512                # moving-operand free dim (one PSUM bank, fp32)
F32 = mybir.dt.float32
BF16 = mybir.dt.bfloat16

MAIN_BANDS = 8                # full bands per core
MAIN_OUT = MAIN_BANDS * BAND_OUT      # 976
MAIN_IN = MAIN_OUT + KH - 1           # 982
HALF_TILES = 8                # col tiles in the half band
HALF_OUT_COLS = HALF_TILES * COL_TILE # 4096
HALF_IN_COLS = HALF_OUT_COLS + 8      # 4104 (6-col halo, padded to 8)

# Results object of the most recent hardware run (for test harnesses).
LAST_RESULTS = None


def _split_multi_waits(nc):
    """Walrus in this toolchain accepts at most ONE sync-wait per
    instruction; Tile's scheduler may attach several.  Hoist the extras onto
    single-wait InstEventSemaphore instructions inserted just before, on the
    same engine stream (a sequence of waits = AND of the conditions)."""
    uid = 0
    for fn in nc.m.functions:
        for blk in fn.blocks:
            new_list = []
            for inst in blk.instructions:
                si = getattr(inst, "sync_info", None)
                if si is not None and si.on_wait and len(si.on_wait) > 1:
                    waits = list(si.on_wait)
                    for w in waits[:-1]:
                        ev = mybir.InstEventSemaphore(
                            name=f"wait_split_{uid}",
                            ins=[],
                            outs=[],
                            sync_info=mybir.SyncInfo(on_wait=[w], on_update=[]),
                        )
                        uid += 1
                        ev.engine = inst.engine
                        new_list.append(ev)
                    si.on_wait = [waits[-1]]
                new_list.append(inst)
            blk.instructions[:] = new_list


def _build_nc(bias_val):
    nc = bass.Bass()
    Xm = nc.declare_dram_parameter("Xm", [MAIN_IN, W], BF16, isOutput=False)
    Xh = nc.declare_dram_parameter("Xh", [BAND_IN, HALF_IN_COLS], BF16, isOutput=False)
    A = nc.declare_dram_parameter("A", [BAND_IN, KW * APAD], BF16, isOutput=False)
    Om = nc.declare_dram_parameter("Om", [MAIN_OUT, OW], BF16, isOutput=True)
    Oh = nc.declare_dram_parameter("Oh", [BAND_OUT, HALF_OUT_COLS], BF16, isOutput=True)

    with TileContext(nc) as tc:
        with (
            tc.tile_pool(name="const", bufs=1) as cpool,
            tc.tile_pool(name="hx", bufs=1) as hxpool,
            tc.tile_pool(name="x", bufs=4) as xpool,
            tc.tile_pool(name="o", bufs=3) as opool,
            tc.tile_pool(name="ps", bufs=8, space="PSUM") as pspool,
        ):
            # PE warm-up: HAM clock-gates the PE to half rate until it has
            # run ~3us continuously.  Real work can't start before the first
            # gating load lands (~10us), so feed the array dummy matmuls on a
            # memset tile from ~7us; by the time real matmuls issue the array
            # is at full clock.  The dummy PSUM bank is never drained.
            dummy = cpool.tile([BAND_IN, COL_TILE], BF16, tag="dummy")
            nc.vector.memset(dummy[:, :], 0.0)
            ps_d = pspool.tile([APAD, COL_TILE], F32, tag="ps")
            for _ in range(8):
                nc.tensor.matmul(
                    ps_d[:, :], lhsT=dummy[:, 0:APAD], rhs=dummy[:, :],
                    start=True, stop=True,
                )

            # Gating loads ride the HWDGE rings (sync/scalar): their queues
            # come up ~3us faster than the gpsimd SWDGE ring and also spread
            # rows across the SDMA engines.  A-halves first, then the two
            # opening half-band pieces, so the first matmul gates on data
            # that lands ~10us in.  Bulk loads ride the gpsimd ring, which
            # has the highest steady-state throughput.
            a_tile = cpool.tile([BAND_IN, KW * APAD], BF16)
            nc.sync.dma_start(out=a_tile[0:64, :], in_=A[0:64, :])
            nc.scalar.dma_start(out=a_tile[64:128, :], in_=A[64:128, :])

            # Opening half-band input, split so col tile 0 gates on a 0.13MB
            # piece: hxa0 = cols 0:520 (sync), hxa1 = cols 512:1544 (scalar).
            hx_a0 = hxpool.tile([BAND_IN, 520], BF16, tag="hxa0")
            hx_a1 = hxpool.tile([BAND_IN, 1032], BF16, tag="hxa1")
            hx_b = hxpool.tile([BAND_IN, HALF_IN_COLS - 1536], BF16, tag="hxb")
            nc.sync.dma_start(out=hx_a0[:, :], in_=Xh[:, 0:520])
            nc.scalar.dma_start(out=hx_a1[:, :], in_=Xh[:, 512:1544])

            # Band 0 loads as four quarter-width tiles so each group of 4
            # col tiles gates on a 0.53 MB piece that lands progressively.
            xt0_q = []
            for k in range(4):
                wq = 2054 if k < 3 else 2048
                t = hxpool.tile([BAND_IN, wq], BF16, tag=f"x0q{k}")
                nc.gpsimd.dma_start(out=t[:, :], in_=Xm[0:BAND_IN, 2048 * k : 2048 * k + wq])
                xt0_q.append(t)

            x_tiles = {}

            def issue_load(bi):
                if bi >= MAIN_BANDS:
                    return
                r0 = bi * BAND_OUT
                xt = xpool.tile([BAND_IN, W], BF16, tag="x")
                nc.gpsimd.dma_start(out=xt[0:64, :], in_=Xm[r0 : r0 + 64, :])
                nc.gpsimd.dma_start(out=xt[64:128, :], in_=Xm[r0 + 64 : r0 + 128, :])
                x_tiles[bi] = xt

            issue_load(1)
            issue_load(2)

            def conv_tile(x_tile, x0, w, o_tile, c0):
                """7 accumulating matmuls into a PSUM bank, drain to o_tile."""
                ps = pspool.tile([APAD, COL_TILE], F32, tag="ps")
                for dx in range(KW):
                    nc.tensor.matmul(
                        ps[:, :w],
                        lhsT=a_tile[:, dx * APAD : (dx + 1) * APAD],
                        rhs=x_tile[:, x0 + dx : x0 + dx + w],
                        start=(dx == 0),
                        stop=(dx == KW - 1),
                    )
                nc.vector.tensor_scalar_add(
                    o_tile[:, c0 : c0 + w], ps[:BAND_OUT, :w], float(bias_val)
                )

            # --- opening 3 half-band col tiles gated only on A + the small
            # hx_a pieces, so the PE starts real work ~10us in.  Their
            # store rides the otherwise-idle HWDGE rings.
            o_ha = opool.tile([BAND_OUT, 3 * COL_TILE], BF16, tag="oha")
            conv_tile(hx_a0, 0, COL_TILE, o_ha, 0)
            for j in range(1, 3):
                conv_tile(hx_a1, (j - 1) * COL_TILE, COL_TILE, o_ha, j * COL_TILE)
            nc.sync.dma_start(out=Oh[0:31, 0:1536], in_=o_ha[0:31, :])
            nc.scalar.dma_start(out=Oh[31:61, 0:1536], in_=o_ha[31:61, :])
            nc.sync.dma_start(out=Oh[61:92, 0:1536], in_=o_ha[61:92, :])
            nc.scalar.dma_start(out=Oh[92:BAND_OUT, 0:1536], in_=o_ha[92:BAND_OUT, :])

            # --- main bands: loads on gpsimd; stores gpsimd-dominant with a
            # 15-row chunk on each HWDGE ring per band.  The FINAL band
            # stores progressively in 4 column chunks (each issued as its 4
            # col tiles drain) so almost nothing is left to flush at the end;
            # mid-stream bands keep whole-band stores (a split there pushes
            # Tile buffer syncs into the PE stream).
            for bi in range(MAIN_BANDS):
                issue_load(bi + 3)
                if bi == 0:
                    nc.gpsimd.dma_start(out=hx_b[:, :], in_=Xh[:, 1536:HALF_IN_COLS])
                o_tile = opool.tile([BAND_OUT, OW], BF16, tag="om")
                s = bi * BAND_OUT
                last = bi == MAIN_BANDS - 1
                x_tile = None if bi == 0 else x_tiles.pop(bi)
                for j in range(16):
                    x0 = j * COL_TILE
                    w = min(COL_TILE, OW - x0)
                    if bi == 0:
                        conv_tile(xt0_q[j // 4], x0 - 2048 * (j // 4), w, o_tile, x0)
                    else:
                        conv_tile(x_tile, x0, w, o_tile, x0)
                    if last and j % 4 == 3:
                        c0, c1 = (j - 3) * COL_TILE, min((j + 1) * COL_TILE, OW)
                        nc.gpsimd.dma_start(
                            out=Om[s : s + BAND_OUT, c0:c1], in_=o_tile[:, c0:c1]
                        )
                if not last:
                    nc.gpsimd.dma_start(out=Om[s : s + 46, :], in_=o_tile[0:46, :])
                    nc.gpsimd.dma_start(out=Om[s + 46 : s + 92, :], in_=o_tile[46:92, :])
                    nc.sync.dma_start(out=Om[s + 92 : s + 107, :], in_=o_tile[92:107, :])
                    nc.scalar.dma_start(out=Om[s + 107 : s + BAND_OUT, :], in_=o_tile[107:BAND_OUT, :])

            # --- closing 5 half-band col tiles: input resident since band
            # 0; their compute hides the last main band's chunk stores, and
            # each tile stores as soon as it drains, so the final flush is
            # three ~40 KB row-slices of the last tile.
            o_hb = opool.tile([BAND_OUT, 5 * COL_TILE], BF16, tag="ohb")
            for j in range(3, HALF_TILES):
                c0 = (j - 3) * COL_TILE
                oc0 = 1536 + c0
                conv_tile(hx_b, j * COL_TILE - 1536, COL_TILE, o_hb, c0)
                if j < HALF_TILES - 1:
                    nc.gpsimd.dma_start(
                        out=Oh[0:BAND_OUT, oc0 : oc0 + COL_TILE],
                        in_=o_hb[:, c0 : c0 + COL_TILE],
                    )
                else:
                    nc.gpsimd.dma_start(
                        out=Oh[0:62, oc0 : oc0 + COL_TILE], in_=o_hb[0:62, c0 : c0 + COL_TILE]
                    )
                    nc.sync.dma_start(
                        out=Oh[62:92, oc0 : oc0 + COL_TILE], in_=o_hb[62:92, c0 : c0 + COL_TILE]
                    )
                    nc.scalar.dma_start(
                        out=Oh[92:BAND_OUT, oc0 : oc0 + COL_TILE],
                        in_=o_hb[92:BAND_OUT, c0 : c0 + COL_TILE],
                    )

    _split_multi_waits(nc)
    return nc


def _make_A(K):
    A = np.zeros((BAND_IN, KW * APAD), np.float32)
    for dx in range(KW):
        for y in range(BAND_OUT):
            A[y : y + KH, dx * APAD + y] = K[:, dx]
    return A.astype(ml_dtypes.bfloat16)


def kernel(X, K, bias, _trace=False):
    global LAST_RESULTS
    X = np.asarray(X, dtype=np.float32)
    K = np.asarray(K, dtype=np.float32)
    bias_val = float(np.asarray(bias).reshape(-1)[0])

    A = _make_A(K)
    Xb = X.astype(ml_dtypes.bfloat16)

    in_maps = []
    for i in range(N_CORES):
        xm = Xb[MAIN_OUT * i : MAIN_OUT * i + MAIN_IN]  # contiguous view
        b = 64 + i // 2
        r0 = BAND_OUT * b
        rows = min(BAND_IN, H - r0)  # band 67 has only 18 real input rows
        xh = np.zeros((BAND_IN, HALF_IN_COLS), ml_dtypes.bfloat16)
        if i % 2 == 0:
            xh[:rows, :] = Xb[r0 : r0 + rows, 0:HALF_IN_COLS]
        else:
            xh[:rows, : W - 4096] = Xb[r0 : r0 + rows, 4096:W]
        in_maps.append({"Xm": xm, "Xh": xh, "A": A})

    nc = _build_nc(bias_val)
    res = run_bass_kernel_spmd(nc, in_maps, core_ids=list(range(N_CORES)), trace=_trace)
    LAST_RESULTS = res

    full = np.empty((OH, OW), np.float32)
    for i in range(N_CORES):
        full[MAIN_OUT * i : MAIN_OUT * (i + 1)] = res.results[i]["Om"].astype(
            np.float32
        )
        b = 64 + i // 2
        r0 = BAND_OUT * b
        nr = min(BAND_OUT, OH - r0)  # band 67: 12 valid rows
        oh = res.results[i]["Oh"].astype(np.float32)
        if i % 2 == 0:
            full[r0 : r0 + nr, 0:4096] = oh[:nr, :4096]
        else:
            full[r0 : r0 + nr, 4096:OW] = oh[:nr, : OW - 4096]
    return full



# revision 7
# speedup vs baseline: 1.0204x; 1.0204x over previous
"""Trainium2 Bass kernel: 7x7 valid 2D cross-correlation of an 8192x8192
fp32 image plus scalar bias, row-sharded across 8 NeuronCores.

Formulation (per core): the y-direction 7-tap convolution for a fixed kernel
column dx is a banded matmul: out_dx[y, x] = sum_r A_dx[r, y] * X[r, x] with
A_dx[r, y] = K[r - y, dx].  The full conv accumulates the 7 dx terms in PSUM
with the moving operand (image columns) shifted by dx.  Matmuls run in bf16
(inputs bf16, fp32 PSUM accumulate); the banded weight blocks are padded to
128 columns so the compiler's fast-weight-load path engages.  This shape is
PE-bound at ~854 useful MACs/cycle (band 7 of a 128-deep contraction), which
is the provable ceiling for conv-as-banded-matmul on this array; fp8
DoubleRow (2x PE rate) was measured numerically and fails the 2e-2 gate
(e4m3 quantization of X alone gives 2.9e-2 max rel err).

Work distribution: 8186 output rows = 68 bands of <=122 rows.  Each core gets
8 full bands (rows 976*i .. 976*i+976) plus HALF of one of bands 64..67
(8 column tiles), i.e. 136 (band, col-tile) units/core instead of the naive
9 full bands = 144 — the PE-time quantum is a full 512-column matmul pass.

DMA plan (from traced ring behavior): the per-queue DGEs serve entries in
order and coalesce CONTIGUOUS DRAM rows into 16 KB packets; non-contiguous
1-4 KB rows degrade to ~50-130 GB/s (measured), so every latency-critical
transfer gets a dedicated contiguous DRAM buffer (host-side re-layout is
free).  Loads and bulk stores ride the gpsimd SWDGE ring (16 SDMA engines);
A and small tail slices ride the sync/scalar HWDGE rings.  Startup: the PE
is clock-gated (HAM) to half rate until ~3us of continuous work, so dummy
matmuls on a memset tile warm it while the first gating loads (A + the first
half-band piece, ~0.4 MB total) land; real matmuls start ~10.5us in at full
clock.  Tail: the final main band stores progressively in 4 contiguous
column chunks as its col tiles drain, and each closing half-band tile stores
as it drains, so the post-compute flush is a ~0.13 MB store + teardown.
"""

import numpy as np
import ml_dtypes

import concourse.bass as bass
import concourse.mybir as mybir
from concourse.tile import TileContext
from concourse.bass_utils import run_bass_kernel_spmd

H = W = 8192
KH = KW = 7
OH = OW = H - KH + 1          # 8186
N_CORES = 8
BAND_IN = 128                 # input rows per matmul band (partition dim)
BAND_OUT = BAND_IN - KH + 1   # 122 output rows per band
APAD = 128                    # A block columns (padded from BAND_OUT for FWL)
COL_TILE = 512                # moving-operand free dim (one PSUM bank, fp32)
F32 = mybir.dt.float32
BF16 = mybir.dt.bfloat16

MAIN_BANDS = 8                # full bands per core
MAIN_OUT = MAIN_BANDS * BAND_OUT      # 976
MAIN_IN = MAIN_OUT + KH - 1           # 982
HALF_TILES = 8                # col tiles in the half band
HALF_OUT_COLS = HALF_TILES * COL_TILE # 4096
HALF_IN_COLS = HALF_OUT_COLS + 8      # 4104 (6-col halo, padded to 8)
Q_W = 2054                    # band-0 quarter width (2048 + 6-col halo)

# band-7 progressive store column chunks (contiguous DRAM targets)
CHUNKS = [(0, 2048), (2048, 4096), (4096, 6144), (6144, OW)]

# Results object of the most recent hardware run (for test harnesses).
LAST_RESULTS = None


def _split_multi_waits(nc):
    """Walrus in this toolchain accepts at most ONE sync-wait per
    instruction; Tile's scheduler may attach several.  Hoist the extras onto
    single-wait InstEventSemaphore instructions inserted just before, on the
    same engine stream (a sequence of waits = AND of the conditions)."""
    uid = 0
    for fn in nc.m.functions:
        for blk in fn.blocks:
            new_list = []
            for inst in blk.instructions:
                si = getattr(inst, "sync_info", None)
                if si is not None and si.on_wait and len(si.on_wait) > 1:
                    waits = list(si.on_wait)
                    for w in waits[:-1]:
                        ev = mybir.InstEventSemaphore(
                            name=f"wait_split_{uid}",
                            ins=[],
                            outs=[],
                            sync_info=mybir.SyncInfo(on_wait=[w], on_update=[]),
                        )
                        uid += 1
                        ev.engine = inst.engine
                        new_list.append(ev)
                    si.on_wait = [waits[-1]]
                new_list.append(inst)
            blk.instructions[:] = new_list


def _build_nc(bias_val):
    nc = bass.Bass()
    Xm = nc.declare_dram_parameter("Xm", [MAIN_IN, W], BF16, isOutput=False)
    Xh0 = nc.declare_dram_parameter("Xh0", [BAND_IN, 520], BF16, isOutput=False)
    Xh1 = nc.declare_dram_parameter("Xh1", [BAND_IN, 1032], BF16, isOutput=False)
    Xhb = nc.declare_dram_parameter("Xhb", [BAND_IN, HALF_IN_COLS - 1536], BF16, isOutput=False)
    Xq = [
        nc.declare_dram_parameter(f"Xq{k}", [BAND_IN, Q_W if k < 3 else 2048], BF16, isOutput=False)
        for k in range(4)
    ]
    A = nc.declare_dram_parameter("A", [BAND_IN, KW * APAD], BF16, isOutput=False)
    # bands 0-6 store full-width rows here; band 7 stores into the four
    # contiguous column-chunk tensors OmL*; the half band stores its opening
    # 3 tiles into Oh (contiguous [122, 1536]) and its closing 5 tiles into
    # per-tile contiguous OhC*.
    Om = nc.declare_dram_parameter("Om", [(MAIN_BANDS - 1) * BAND_OUT, OW], BF16, isOutput=True)
    OmL = [
        nc.declare_dram_parameter(f"OmL{k}", [BAND_OUT, c1 - c0], BF16, isOutput=True)
        for k, (c0, c1) in enumerate(CHUNKS)
    ]
    Oh = nc.declare_dram_parameter("Oh", [BAND_OUT, 3 * COL_TILE], BF16, isOutput=True)
    OhC = [
        nc.declare_dram_parameter(f"OhC{k}", [BAND_OUT, COL_TILE], BF16, isOutput=True)
        for k in range(5)
    ]

    with TileContext(nc) as tc:
        with (
            tc.tile_pool(name="const", bufs=1) as cpool,
            tc.tile_pool(name="hx", bufs=1) as hxpool,
            tc.tile_pool(name="x", bufs=4) as xpool,
            tc.tile_pool(name="o", bufs=3) as opool,
            tc.tile_pool(name="ps", bufs=8, space="PSUM") as pspool,
        ):
            # PE warm-up: HAM clock-gates the PE to half rate until it has
            # run ~3us continuously.  Real work can't start before the first
            # gating loads land (~10.5us), so feed the array dummy matmuls on
            # a memset tile from ~8.3us; the dummy PSUM writes are never read.
            dummy = cpool.tile([BAND_IN, COL_TILE], BF16, tag="dummy")
            nc.vector.memset(dummy[:, :], 0.0)
            ps_d = pspool.tile([APAD, COL_TILE], F32, tag="ps")
            for _ in range(5):
                nc.tensor.matmul(
                    ps_d[:, :], lhsT=dummy[:, 0:APAD], rhs=dummy[:, :],
                    start=True, stop=True,
                )

            # A rides the HWDGE rings (fast from idle, done ~9.7us); the
            # gating half-band pieces are the gpsimd ring's FIRST entries
            # (in-order service => priority over the bulk), each from a
            # dedicated contiguous DRAM buffer so the DGE coalesces 16KB
            # packets.
            a_tile = cpool.tile([BAND_IN, KW * APAD], BF16)
            nc.sync.dma_start(out=a_tile[0:64, :], in_=A[0:64, :])
            nc.scalar.dma_start(out=a_tile[64:128, :], in_=A[64:128, :])

            hx_a0 = hxpool.tile([BAND_IN, 520], BF16, tag="hxa0")
            hx_a1 = hxpool.tile([BAND_IN, 1032], BF16, tag="hxa1")
            hx_b = hxpool.tile([BAND_IN, HALF_IN_COLS - 1536], BF16, tag="hxb")
            nc.gpsimd.dma_start(out=hx_a0[:, :], in_=Xh0[:, :])
            nc.gpsimd.dma_start(out=hx_a1[:, :], in_=Xh1[:, :])

            # Band 0 loads as four contiguous quarter-width buffers so each
            # group of 4 col tiles gates on a 0.53 MB piece that lands
            # progressively.
            xt0_q = []
            for k in range(4):
                wq = Q_W if k < 3 else 2048
                t = hxpool.tile([BAND_IN, wq], BF16, tag=f"x0q{k}")
                nc.gpsimd.dma_start(out=t[:, :], in_=Xq[k][:, :])
                xt0_q.append(t)

            x_tiles = {}

            def issue_load(bi):
                if bi >= MAIN_BANDS:
                    return
                r0 = bi * BAND_OUT
                xt = xpool.tile([BAND_IN, W], BF16, tag="x")
                nc.gpsimd.dma_start(out=xt[0:64, :], in_=Xm[r0 : r0 + 64, :])
                nc.gpsimd.dma_start(out=xt[64:128, :], in_=Xm[r0 + 64 : r0 + 128, :])
                x_tiles[bi] = xt

            issue_load(1)
            issue_load(2)

            def conv_tile(x_tile, x0, w, o_tile, c0):
                """7 accumulating matmuls into a PSUM bank, drain to o_tile."""
                ps = pspool.tile([APAD, COL_TILE], F32, tag="ps")
                for dx in range(KW):
                    nc.tensor.matmul(
                        ps[:, :w],
                        lhsT=a_tile[:, dx * APAD : (dx + 1) * APAD],
                        rhs=x_tile[:, x0 + dx : x0 + dx + w],
                        start=(dx == 0),
                        stop=(dx == KW - 1),
                    )
                nc.vector.tensor_scalar_add(
                    o_tile[:, c0 : c0 + w], ps[:BAND_OUT, :w], float(bias_val)
                )

            # --- opening 3 half-band col tiles gated only on A + the two
            # small hx pieces, so the PE starts real work ~10.5us in.  Their
            # store rides the otherwise-idle HWDGE rings (Oh is contiguous).
            o_ha = opool.tile([BAND_OUT, 3 * COL_TILE], BF16, tag="oha")
            conv_tile(hx_a0, 0, COL_TILE, o_ha, 0)
            for j in range(1, 3):
                conv_tile(hx_a1, (j - 1) * COL_TILE, COL_TILE, o_ha, j * COL_TILE)
            nc.sync.dma_start(out=Oh[0:31, :], in_=o_ha[0:31, :])
            nc.scalar.dma_start(out=Oh[31:61, :], in_=o_ha[31:61, :])
            nc.sync.dma_start(out=Oh[61:92, :], in_=o_ha[61:92, :])
            nc.scalar.dma_start(out=Oh[92:BAND_OUT, :], in_=o_ha[92:BAND_OUT, :])

            # --- main bands: loads on gpsimd; stores gpsimd-dominant with a
            # 15-row chunk on each HWDGE ring per band.  The FINAL band
            # stores progressively into contiguous column-chunk tensors as
            # its col tiles drain, so almost nothing is left to flush at the
            # end.
            for bi in range(MAIN_BANDS):
                issue_load(bi + 3)
                if bi == 0:
                    nc.gpsimd.dma_start(out=hx_b[:, :], in_=Xhb[:, :])
                o_tile = opool.tile([BAND_OUT, OW], BF16, tag="om")
                s = bi * BAND_OUT
                last = bi == MAIN_BANDS - 1
                x_tile = None if bi == 0 else x_tiles.pop(bi)
                for j in range(16):
                    x0 = j * COL_TILE
                    w = min(COL_TILE, OW - x0)
                    if bi == 0:
                        conv_tile(xt0_q[j // 4], x0 - 2048 * (j // 4), w, o_tile, x0)
                    else:
                        conv_tile(x_tile, x0, w, o_tile, x0)
                    if last and j % 4 == 3:
                        k = j // 4
                        c0, c1 = CHUNKS[k]
                        nc.gpsimd.dma_start(out=OmL[k][:, :], in_=o_tile[:, c0:c1])
                if not last:
                    nc.gpsimd.dma_start(out=Om[s : s + 46, :], in_=o_tile[0:46, :])
                    nc.gpsimd.dma_start(out=Om[s + 46 : s + 92, :], in_=o_tile[46:92, :])
                    nc.sync.dma_start(out=Om[s + 92 : s + 107, :], in_=o_tile[92:107, :])
                    nc.scalar.dma_start(out=Om[s + 107 : s + BAND_OUT, :], in_=o_tile[107:BAND_OUT, :])

            # --- closing 5 half-band col tiles: input resident since band
            # 0; their compute hides the last main band's chunk stores, and
            # each tile stores (contiguously) as soon as it drains, so the
            # final flush is three small row-slices of the last tile.
            o_hb = opool.tile([BAND_OUT, 5 * COL_TILE], BF16, tag="ohb")
            for j in range(3, HALF_TILES):
                c0 = (j - 3) * COL_TILE
                conv_tile(hx_b, j * COL_TILE - 1536, COL_TILE, o_hb, c0)
                t = OhC[j - 3]
                if j < HALF_TILES - 1:
                    nc.gpsimd.dma_start(out=t[:, :], in_=o_hb[:, c0 : c0 + COL_TILE])
                else:
                    nc.gpsimd.dma_start(out=t[0:62, :], in_=o_hb[0:62, c0 : c0 + COL_TILE])
                    nc.sync.dma_start(out=t[62:92, :], in_=o_hb[62:92, c0 : c0 + COL_TILE])
                    nc.scalar.dma_start(out=t[92:BAND_OUT, :], in_=o_hb[92:BAND_OUT, c0 : c0 + COL_TILE])

    _split_multi_waits(nc)
    return nc


def _make_A(K):
    A = np.zeros((BAND_IN, KW * APAD), np.float32)
    for dx in range(KW):
        for y in range(BAND_OUT):
            A[y : y + KH, dx * APAD + y] = K[:, dx]
    return A.astype(ml_dtypes.bfloat16)


def kernel(X, K, bias, _trace=False):
    global LAST_RESULTS
    X = np.asarray(X, dtype=np.float32)
    K = np.asarray(K, dtype=np.float32)
    bias_val = float(np.asarray(bias).reshape(-1)[0])

    A = _make_A(K)
    Xb = X.astype(ml_dtypes.bfloat16)

    in_maps = []
    for i in range(N_CORES):
        xm = Xb[MAIN_OUT * i : MAIN_OUT * i + MAIN_IN]  # contiguous view
        b = 64 + i // 2
        r0 = BAND_OUT * b
        rows = min(BAND_IN, H - r0)  # band 67 has only 18 real input rows
        xh = np.zeros((BAND_IN, HALF_IN_COLS), ml_dtypes.bfloat16)
        if i % 2 == 0:
            xh[:rows, :] = Xb[r0 : r0 + rows, 0:HALF_IN_COLS]
        else:
            xh[:rows, : W - 4096] = Xb[r0 : r0 + rows, 4096:W]
        im = {
            "Xm": xm,
            "Xh0": np.ascontiguousarray(xh[:, 0:520]),
            "Xh1": np.ascontiguousarray(xh[:, 512:1544]),
            "Xhb": np.ascontiguousarray(xh[:, 1536:HALF_IN_COLS]),
            "A": A,
        }
        for k in range(4):
            wq = Q_W if k < 3 else 2048
            im[f"Xq{k}"] = np.ascontiguousarray(xm[0:BAND_IN, 2048 * k : 2048 * k + wq])
        in_maps.append(im)

    nc = _build_nc(bias_val)
    res = run_bass_kernel_spmd(nc, in_maps, core_ids=list(range(N_CORES)), trace=_trace)
    LAST_RESULTS = res

    full = np.empty((OH, OW), np.float32)
    for i in range(N_CORES):
        r = res.results[i]
        base = MAIN_OUT * i
        full[base : base + (MAIN_BANDS - 1) * BAND_OUT] = r["Om"].astype(np.float32)
        s7 = base + (MAIN_BANDS - 1) * BAND_OUT
        for k, (c0, c1) in enumerate(CHUNKS):
            full[s7 : s7 + BAND_OUT, c0:c1] = r[f"OmL{k}"].astype(np.float32)
        # half band
        b = 64 + i // 2
        r0 = BAND_OUT * b
        nr = min(BAND_OUT, OH - r0)  # band 67: 12 valid rows
        oh = np.concatenate(
            [r["Oh"]] + [r[f"OhC{k}"] for k in range(5)], axis=1
        ).astype(np.float32)
        if i % 2 == 0:
            full[r0 : r0 + nr, 0:4096] = oh[:nr, :4096]
        else:
            full[r0 : r0 + nr, 4096:OW] = oh[:nr, : OW - 4096]
    return full
